# revision 3
# baseline (speedup 1.0000x reference)
"""Trainium2 Bass kernel for BinaryLinear: out = x @ sign(W).T

Shapes (hardcoded): x [32768, 2048] f32, weight [2048, 2048] f32,
out [32768, 2048] f32.

Strategy: data-parallel over 8 NeuronCores — shard the token axis
(4096 tokens/core), replicate the weight. Per core:
  - weight prep: DMA W, sign via ScalarE (f32 -> bf16), PE-transpose
    128x128 blocks to build sign(W).T ("swT") resident in SBUF as bf16.
  - main loop over 32 token tiles of 128: DMA x tile, cast f32->bf16
    (DVE), PE-transpose the 16 [128,128] i-chunks to get x.T stationary
    operands, then accumulate out[128t, 512o] = sum_ic xT_ic.T @ swT_ic
    over 16 i-chunks x 4 o-chunks into 4 PSUM banks, copy out via DVE,
    DMA the [128, 2048] f32 tile back to DRAM.

bf16 matmul with f32 PSUM accumulation: weights are exactly +-1 in
bf16; only x rounds to bf16 (~1e-3 relative output error).
"""

import sys

if "/opt/trn_rl_repo" not in sys.path:
    sys.path.insert(0, "/opt/trn_rl_repo")

import numpy as np

T, I, O = 32768, 2048, 2048
NCORES = 8
TL = T // NCORES  # tokens per core

_NC = None


def _build():
    import concourse.bacc as bacc
    import concourse.mybir as mybir
    from concourse import masks, tile
    from contextlib import ExitStack

    f32 = mybir.dt.float32
    bf16 = mybir.dt.bfloat16

    IC = I // 128  # i-chunks (contraction)
    OT = O // 128  # weight row tiles
    NT = TL // 128  # token tiles per core
    OCW = 512  # matmul moving free dim
    NOC = O // OCW

    nc = bacc.Bacc("TRN2", target_bir_lowering=False, debug=False, num_devices=NCORES)
    x = nc.dram_tensor("x", [TL, I], f32, kind="ExternalInput")
    w = nc.dram_tensor("weight", [O, I], f32, kind="ExternalInput")
    out = nc.dram_tensor("out", [TL, O], f32, kind="ExternalOutput")

    with tile.TileContext(nc) as tc, ExitStack() as ctx:
        const_pool = ctx.enter_context(tc.tile_pool(name="const", bufs=1))
        ident = const_pool.tile([128, 128], bf16)
        masks.make_identity(nc, ident[:])

        # sign(W).T, bf16, resident: IC tiles of [128 i, O]
        swt_pool = ctx.enter_context(tc.tile_pool(name="swt", bufs=1))
        swT = [swt_pool.tile([128, O], bf16, name=f"swT{ic}") for ic in range(IC)]

        wprep = ctx.enter_context(tc.tile_pool(name="wprep", bufs=2))
        psum_tr = ctx.enter_context(tc.tile_pool(name="psum_tr", bufs=3, space="PSUM"))

        for ot in range(OT):
            w_f32 = wprep.tile([128, I], f32, tag="w_f32")
            nc.sync.dma_start(w_f32[:], w[128 * ot : 128 * (ot + 1), :])
            w_sgn = wprep.tile([128, I], bf16, tag="w_sgn")
            nc.scalar.activation(
                w_sgn[:], w_f32[:], mybir.ActivationFunctionType.Sign
            )
            for ic in range(IC):
                ps = psum_tr.tile([128, 128], bf16, tag="ps_tr")
                nc.tensor.transpose(ps[:], w_sgn[:, 128 * ic : 128 * (ic + 1)], ident[:])
                if ic % 2 == 0:
                    nc.vector.tensor_copy(swT[ic][:, 128 * ot : 128 * (ot + 1)], ps[:])
                else:
                    nc.scalar.copy(swT[ic][:, 128 * ot : 128 * (ot + 1)], ps[:])

        xpool = ctx.enter_context(tc.tile_pool(name="xpool", bufs=3))
        xtpool = ctx.enter_context(tc.tile_pool(name="xtpool", bufs=2))
        opool = ctx.enter_context(tc.tile_pool(name="opool", bufs=3))
        psum_mm = ctx.enter_context(tc.tile_pool(name="psum_mm", bufs=1, space="PSUM"))

        for tt in range(NT):
            x_f32 = xpool.tile([128, I], f32, tag="x_f32")
            nc.sync.dma_start(x_f32[:], x[128 * tt : 128 * (tt + 1), :])
            x_bf = xpool.tile([128, I], bf16, tag="x_bf")
            nc.vector.tensor_copy(x_bf[:], x_f32[:])

            # transpose x tile: xT[:, ic, :] = x_bf[:, ic-chunk].T
            xT = xtpool.tile([128, IC, 128], bf16, tag="xT")
            for ic in range(IC):
                ps = psum_tr.tile([128, 128], bf16, tag="ps_tr")
                nc.tensor.transpose(ps[:], x_bf[:, 128 * ic : 128 * (ic + 1)], ident[:])
                if ic % 2 == 0:
                    nc.vector.tensor_copy(xT[:, ic, :], ps[:])
                else:
                    nc.scalar.copy(xT[:, ic, :], ps[:])

            accs = [
                psum_mm.tile([128, OCW], f32, tag=f"acc{oc}", name=f"acc{oc}")
                for oc in range(NOC)
            ]
            for ic in range(IC):
                for oc in range(NOC):
                    nc.tensor.matmul(
                        accs[oc][:],
                        xT[:, ic, :],
                        swT[ic][:, OCW * oc : OCW * (oc + 1)],
                        start=(ic == 0),
                        stop=(ic == IC - 1),
                    )

            o_sb = opool.tile([128, O], f32, tag="o_sb")
            for oc in range(NOC):
                nc.vector.tensor_copy(o_sb[:, OCW * oc : OCW * (oc + 1)], accs[oc][:])
            nc.sync.dma_start(out[128 * tt : 128 * (tt + 1), :], o_sb[:])

    nc.compile()
    return nc


def _get_nc():
    global _NC
    if _NC is None:
        _NC = _build()
    return _NC


def _in_maps(x, w):
    x = np.ascontiguousarray(np.asarray(x, dtype=np.float32))
    w = np.ascontiguousarray(np.asarray(w, dtype=np.float32))
    assert x.shape == (T, I) and w.shape == (O, I)
    return [
        {"x": x[c * TL : (c + 1) * TL], "weight": w} for c in range(NCORES)
    ]


def kernel(**inputs):
    from concourse.bass_utils import run_bass_kernel_spmd

    nc = _get_nc()
    res = run_bass_kernel_spmd(
        nc, _in_maps(inputs["x"], inputs["weight"]), core_ids=list(range(NCORES))
    )
    return np.concatenate([r["out"] for r in res.results], axis=0)


# revision 5
# speedup vs baseline: 1.0303x; 1.0303x over previous
"""Trainium2 Bass kernel for BinaryLinear: out = x @ sign(W).T

Shapes (hardcoded): x [32768, 2048] f32, weight [2048, 2048] f32,
out [32768, 2048] f32.

Strategy: data-parallel over 8 NeuronCores — shard the token axis
(4096 tokens/core), replicate the weight. Per core:
  - weight prep: DMA W, sign via ScalarE (f32 -> bf16), batched xbar
    DMA-transpose into swT[i-part, ic, ot, o] bf16 resident in SBUF.
  - main loop over 32 token tiles of 128: DMA x tile (f32), cast
    f32->bf16 (DVE), one batched xbar DMA-transpose -> xT[i, ic, t],
    then accumulate out[128t, 512o] over 16 i-chunks x 4 o-chunks of
    bf16 matmuls (xT chunks stationary, swT moving) into 4 PSUM banks
    (double-buffered), DVE-copy PSUM->SBUF, DMA the f32 tile out.

The tensor engine runs only the 2048 N=512 matmuls per core
(~213 ns each warm); all transposes ride the DMA xbar.
"""

import sys

if "/opt/trn_rl_repo" not in sys.path:
    sys.path.insert(0, "/opt/trn_rl_repo")

import numpy as np

T, I, O = 32768, 2048, 2048
NCORES = 8
TL = T // NCORES  # tokens per core

_NC = None


def _build():
    import concourse.bacc as bacc
    import concourse.mybir as mybir
    from concourse import tile
    from contextlib import ExitStack

    f32 = mybir.dt.float32
    bf16 = mybir.dt.bfloat16

    IC = I // 128  # i-chunks (contraction)
    OT = O // 128  # weight row tiles
    NT = TL // 128  # token tiles per core
    OCW = 512  # matmul moving free dim
    NOC = O // OCW

    nc = bacc.Bacc("TRN2", target_bir_lowering=False, debug=False, num_devices=NCORES)
    x = nc.dram_tensor("x", [TL, I], f32, kind="ExternalInput")
    w = nc.dram_tensor("weight", [O, I], f32, kind="ExternalInput")
    out = nc.dram_tensor("out", [TL, O], f32, kind="ExternalOutput")

    with tile.TileContext(nc) as tc, ExitStack() as ctx:
        # sign(W).T resident in SBUF: swT[i_p, ic, ot, o_l] =
        # sign(W)[128*ot + o_l, 128*ic + i_p]
        swt_pool = ctx.enter_context(tc.tile_pool(name="swt", bufs=1))
        swT = swt_pool.tile([128, IC, OT, 128], bf16)

        wprep = ctx.enter_context(tc.tile_pool(name="wprep", bufs=2))
        for ot in range(OT):
            w_f32 = wprep.tile([128, I], f32, tag="w_f32")
            nc.sync.dma_start(w_f32[:], w[128 * ot : 128 * (ot + 1), :])
            w_sgn = wprep.tile([128, I], bf16, tag="w_sgn")
            nc.scalar.activation(
                w_sgn[:], w_f32[:], mybir.ActivationFunctionType.Sign
            )
            # out[p, m, l] = in[l, 128m + p]: one batched xbar transpose
            nc.sync.dma_start(swT[:, :, ot, :], w_sgn[:], transpose=True)

        xpool = ctx.enter_context(tc.tile_pool(name="xpool", bufs=3))
        xtpool = ctx.enter_context(tc.tile_pool(name="xtpool", bufs=3))
        opool = ctx.enter_context(tc.tile_pool(name="opool", bufs=3))
        psum_mm = ctx.enter_context(tc.tile_pool(name="psum_mm", bufs=2, space="PSUM"))

        for tt in range(NT):
            x_f32 = xpool.tile([128, I], f32, tag="x_f32")
            nc.sync.dma_start(x_f32[:], x[128 * tt : 128 * (tt + 1), :])
            x_bf = xpool.tile([128, I], bf16, tag="x_bf")
            nc.vector.tensor_copy(x_bf[:], x_f32[:])

            # xT[:, ic, :] = x_bf[:, 128*ic : 128*(ic+1)].T in one shot
            xT = xtpool.tile([128, IC, 128], bf16, tag="xT")
            nc.sync.dma_start(xT[:], x_bf[:], transpose=True)

            accs = [
                psum_mm.tile([128, OCW], f32, tag=f"acc{oc}", name=f"acc{oc}")
                for oc in range(NOC)
            ]
            for ic in range(IC):
                for oc in range(NOC):
                    nc.tensor.matmul(
                        accs[oc][:],
                        xT[:, ic, :],
                        swT[:, ic, 4 * oc : 4 * (oc + 1), :],
                        start=(ic == 0),
                        stop=(ic == IC - 1),
                    )

            o_sb = opool.tile([128, O], f32, tag="o_sb")
            for oc in range(NOC):
                nc.vector.tensor_copy(o_sb[:, OCW * oc : OCW * (oc + 1)], accs[oc][:])
            nc.sync.dma_start(out[128 * tt : 128 * (tt + 1), :], o_sb[:])

    nc.compile()
    return nc


def _get_nc():
    global _NC
    if _NC is None:
        _NC = _build()
    return _NC


def _in_maps(x, w):
    x = np.ascontiguousarray(np.asarray(x, dtype=np.float32))
    w = np.ascontiguousarray(np.asarray(w, dtype=np.float32))
    assert x.shape == (T, I) and w.shape == (O, I)
    return [
        {"x": x[c * TL : (c + 1) * TL], "weight": w} for c in range(NCORES)
    ]


def kernel(**inputs):
    from concourse.bass_utils import run_bass_kernel_spmd

    nc = _get_nc()
    res = run_bass_kernel_spmd(
        nc, _in_maps(inputs["x"], inputs["weight"]), core_ids=list(range(NCORES))
    )
    return np.concatenate([r["out"] for r in res.results], axis=0)


# revision 6
# speedup vs baseline: 1.0452x; 1.0145x over previous
"""Trainium2 Bass kernel for BinaryLinear: out = x @ sign(W).T

Shapes (hardcoded): x [32768, 2048] f32, weight [2048, 2048] f32,
out [32768, 2048] f32.

Strategy: data-parallel over 8 NeuronCores — shard the token axis
(4096 tokens/core), replicate the weight. Per core:
  - weight prep: DMA W via GpSimd queue (keeps the Sync queue free for
    transposes), sign via ScalarE (f32 -> bf16), batched xbar
    DMA-transpose into swT[i-part, ic, ot, o] bf16 resident in SBUF.
  - main loop over 32 token tiles of 128: DMA x tile (f32), cast
    f32->bf16 (DVE), one batched xbar DMA-transpose -> xT[i, ic, t],
    then accumulate out[128t, 512o] per o-chunk (o-chunk outer, so
    early tiles depend only on early weight tiles) over 16 i-chunks of
    bf16 matmuls into PSUM, DVE-copy PSUM->SBUF, DMA the f32 tile out.

All DMA transposes are issued from the Sync engine only — concurrent
DMA_TRANSPOSE from two engines corrupts the shared xbar.
"""

import sys

if "/opt/trn_rl_repo" not in sys.path:
    sys.path.insert(0, "/opt/trn_rl_repo")

import numpy as np

T, I, O = 32768, 2048, 2048
NCORES = 8
TL = T // NCORES  # tokens per core

_NC = None


def _build():
    import concourse.bacc as bacc
    import concourse.mybir as mybir
    from concourse import tile
    from contextlib import ExitStack

    f32 = mybir.dt.float32
    bf16 = mybir.dt.bfloat16

    IC = I // 128  # i-chunks (contraction)
    OT = O // 128  # weight row tiles
    NT = TL // 128  # token tiles per core
    OCW = 512  # matmul moving free dim
    NOC = O // OCW

    nc = bacc.Bacc("TRN2", target_bir_lowering=False, debug=False, num_devices=NCORES)
    x = nc.dram_tensor("x", [TL, I], f32, kind="ExternalInput")
    w = nc.dram_tensor("weight", [O, I], f32, kind="ExternalInput")
    out = nc.dram_tensor("out", [TL, O], f32, kind="ExternalOutput")

    with tile.TileContext(nc) as tc, ExitStack() as ctx:
        # sign(W).T resident in SBUF: swT[i_p, ic, ot, o_l] =
        # sign(W)[128*ot + o_l, 128*ic + i_p]
        swt_pool = ctx.enter_context(tc.tile_pool(name="swt", bufs=1))
        swT = swt_pool.tile([128, IC, OT, 128], bf16)

        wprep = ctx.enter_context(tc.tile_pool(name="wprep", bufs=4))
        w_tiles = []
        for ot in range(OT):
            w_f32 = wprep.tile([128, I], f32, tag="w_f32", name=f"w_f32_{ot}")
            nc.gpsimd.dma_start(w_f32[:], w[128 * ot : 128 * (ot + 1), :])
            w_tiles.append(w_f32)
        for ot in range(OT):
            w_sgn = wprep.tile([128, I], bf16, tag="w_sgn", name=f"w_sgn_{ot}")
            nc.scalar.activation(
                w_sgn[:], w_tiles[ot][:], mybir.ActivationFunctionType.Sign
            )
            # out[p, m, l] = in[l, 128m + p]: one batched xbar transpose
            nc.sync.dma_start(swT[:, :, ot, :], w_sgn[:], transpose=True)

        xpool = ctx.enter_context(tc.tile_pool(name="xpool", bufs=3))
        xtpool = ctx.enter_context(tc.tile_pool(name="xtpool", bufs=3))
        opool = ctx.enter_context(tc.tile_pool(name="opool", bufs=3))
        psum_mm = ctx.enter_context(tc.tile_pool(name="psum_mm", bufs=4, space="PSUM"))

        for tt in range(NT):
            x_f32 = xpool.tile([128, I], f32, tag="x_f32")
            nc.sync.dma_start(x_f32[:], x[128 * tt : 128 * (tt + 1), :])
            x_bf = xpool.tile([128, I], bf16, tag="x_bf")
            nc.vector.tensor_copy(x_bf[:], x_f32[:])

            # xT[:, ic, :] = x_bf[:, 128*ic : 128*(ic+1)].T in one shot
            xT = xtpool.tile([128, IC, 128], bf16, tag="xT")
            nc.sync.dma_start(xT[:], x_bf[:], transpose=True)

            o_sb = opool.tile([128, O], f32, tag="o_sb")
            for oc in range(NOC):
                acc = psum_mm.tile([128, OCW], f32, tag="acc", name=f"acc{tt}_{oc}")
                for ic in range(IC):
                    nc.tensor.matmul(
                        acc[:],
                        xT[:, ic, :],
                        swT[:, ic, 4 * oc : 4 * (oc + 1), :],
                        start=(ic == 0),
                        stop=(ic == IC - 1),
                    )
                nc.vector.tensor_copy(o_sb[:, OCW * oc : OCW * (oc + 1)], acc[:])
            nc.sync.dma_start(out[128 * tt : 128 * (tt + 1), :], o_sb[:])

    nc.compile()
    return nc


def _get_nc():
    global _NC
    if _NC is None:
        _NC = _build()
    return _NC


def _in_maps(x, w):
    x = np.ascontiguousarray(np.asarray(x, dtype=np.float32))
    w = np.ascontiguousarray(np.asarray(w, dtype=np.float32))
    assert x.shape == (T, I) and w.shape == (O, I)
    return [
        {"x": x[c * TL : (c + 1) * TL], "weight": w} for c in range(NCORES)
    ]


def kernel(**inputs):
    from concourse.bass_utils import run_bass_kernel_spmd

    nc = _get_nc()
    res = run_bass_kernel_spmd(
        nc, _in_maps(inputs["x"], inputs["weight"]), core_ids=list(range(NCORES))
    )
    return np.concatenate([r["out"] for r in res.results], axis=0)
